# revision 1
# baseline (speedup 1.0000x reference)
"""MoE layer kernel for 8x TRN2 NeuronCores (Bass/Tile).

Math (reference):
    w      = softmax(x @ gate_W + gate_b, axis=-1)[:E]          # [E, F]
    W_eff  = einsum('ef,edf->df', w, expert_W)                  # [D, F]
    b_eff  = einsum('ef,ef->f',  w, expert_b)                   # [F]
    out    = x @ W_eff + b_eff                                  # [N, F]

Sharding: F-parallel across 8 cores (each core owns 128 f-columns).
  - gate_W/gate_b are column-rolled per core so the shard is columns 0:128
    (the softmax row-sum is order invariant, so rolling columns is harmless).
  - expert_W[:, :, shard] is pre-transposed on the host to [f, d, e] layout
    (pure input marshalling; it makes the expert axis contiguous so the
    weighted reduction runs on the vector engine at full rate).
  - expert_b[:, shard] transposed to [f, e].
  - Each core computes out[:, shard].T as [128, 4096]; the host transposes
    and concatenates.

Device algorithm per core:
  1. Load x in [128, 256] chunks, build xT [2][128, 4096] via PE transposes.
  2. Gate GEMM (tokens 0..1023) + bias + exp (+row-sum via accum_out) +
     normalize -> w_norm [e_p, a, f]; transpose to wnT [f_p, e=1024].
  3. b_eff shard via ONE scalar_tensor_tensor with accum_out (reduce over
     e on the free axis) -> beff_col [f_p, 1].
  4. W_eff shard on the VECTOR engine: stream expert_W tiles
     [f=128p, d=8, e=1024] (4KB-contiguous chunks, dual HWDGE rings); for
     each d: scalar_tensor_tensor(junk = tile[:, d, :] * wnT,
     accum_out=W_effT[:, d]) — a 1024-wide multiply-reduce per partition.
     (The PE matvec formulation measured ~630 ns per LDW+MM — instruction
     overhead bound; this DVE form measures ~1.5 us per 1024-wide op,
     ~2.3x faster per pass, and leaves the PE idle.)
  5. Transpose W_effT -> W_eff [d_p, f]; out^T = W_eff^T @ x^T on PE;
     bias fused into the ACT psum->SBUF copy (per-partition bias);
     chunked DMA out.

NOTE: this walrus build rejects any instruction carrying more than ONE
semaphore wait ("Too many sync wait commands"). _split_multi_waits()
post-processes the scheduled program, hoisting extra waits onto standalone
EventSemaphore instructions on the same engine queue (the same primitive
Tile's own barriers use). Cheap "touch" matmuls / ACT copies still absorb
predictable ticks early so hot-path instructions rarely need the split.
"""

import numpy as np

N, D, E, F = 4096, 256, 1024, 1024
NCORES = 8
FSH = F // NCORES  # 128 f-columns per core
P = 128

_CACHE = {}
LAST_RESULT = None


def _split_multi_waits(nc):
    """Split multi-wait instructions into chains of single-wait ones."""
    import concourse.mybir as mybir

    n = 0
    for fn in nc.m.functions:
        for bb in fn.blocks:
            out = []
            changed = False
            for ins in bb.instructions:
                si = ins.sync_info
                if si is not None and si.on_wait and len(si.on_wait) > 1:
                    waits = list(si.on_wait)
                    for w in waits[:-1]:
                        es = mybir.InstEventSemaphore(
                            name=f"wsplit_{n}",
                            engine=ins.engine,
                            sync_info=mybir.SyncInfo(
                                on_wait=[w], on_update=[]),
                        )
                        out.append(es)
                        n += 1
                    ins.sync_info = mybir.SyncInfo(
                        on_wait=[waits[-1]], on_update=list(si.on_update))
                    changed = True
                out.append(ins)
            if changed:
                bb.instructions = out
    return n


def _build_bass(repeats=1, wet_bufs=3, d_chunk=8):
    import concourse.bass as bass
    import concourse.mybir as mybir
    from concourse.masks import make_identity
    from concourse.tile import TileContext

    f32 = mybir.dt.float32
    AF = mybir.ActivationFunctionType
    mult = mybir.AluOpType.mult

    nc = bass.Bass(trn_type="TRN2", name="moe_fshard",
                   dynamic_dma_scratch_size=4096)

    x_d = nc.dram_tensor("x", [N, D], f32, kind="ExternalInput")
    gw_d = nc.dram_tensor("gw", [D, F], f32, kind="ExternalInput")
    gb_d = nc.dram_tensor("gb", [1, F], f32, kind="ExternalInput")
    # expert_W shard, host-transposed to [f, d, e]
    ew_d = nc.dram_tensor("ewt", [FSH, D, E], f32, kind="ExternalInput")
    # expert_b shard, host-transposed to [f, e]
    eb_d = nc.dram_tensor("ebt", [FSH, E], f32, kind="ExternalInput")
    out_d = nc.dram_tensor("outT", [FSH, N], f32, kind="ExternalOutput")

    EBLK = E // P       # 8 gate token tiles
    TTILE = N // P      # 32 token tiles for the x transpose
    DH = D // P         # 2 halves of d
    NDT = D // d_chunk  # wet tiles per pass

    with TileContext(nc) as tc:
        with tc.tile_pool(name="persist", bufs=1) as persist, \
             tc.tile_pool(name="xcp", bufs=2) as xcp, \
             tc.tile_pool(name="wep", bufs=wet_bufs) as wep, \
             tc.tile_pool(name="dummyp", bufs=1, space="PSUM") as dummyp:

            # smalls packs tiny constants:
            #  [:,0:128] identity; row0 129:257 ones_r128;
            #  [:,257:258] beff_col; row0 258:259 act_scratch;
            #  row0 1024:2048 gate_b
            smalls = persist.tile([P, 2048], f32)
            ident = smalls[:, 0:128]
            ones_r128 = smalls[0:1, 129:257]
            beff_col = smalls[:, 257:258]
            act_scr = smalls[0:1, 258:259]
            gb_sb = smalls[0:1, 1024:2048]

            make_identity(nc, ident)
            nc.vector.memset(smalls[:, 128:130], 1.0)

            xT = persist.tile([P, DH, N], f32)          # 32KB/part
            wnorm = persist.tile([P, EBLK, FSH], f32)   # 4KB/part
            wnT = persist.tile([P, E], f32)             # 4KB/part
            wefft = persist.tile([P, D], f32)           # 1KB/part
            weff = persist.tile([P, DH * FSH], f32)     # 1KB/part
            scr = persist.tile([P, 2], f32)             # rsum, rcp
            rsum = scr[:, 0:1]
            rcp = scr[:, 1:2]
            expsc = persist.tile([P, F], f32)           # 4KB/part
            junk = persist.tile([P, E], f32)            # 4KB/part
            outT_sb = persist.tile([P, N], f32)         # 16KB/part
            gw_sb = persist.tile([P, DH, F], f32)       # 8KB/part
            ebT_sb = persist.tile([P, E], f32)          # 4KB/part

            dummy = dummyp.tile([1, 1], f32)

            # ---- input DMAs (no deps)
            nc.sync.dma_start(
                out=gw_sb[:], in_=gw_d.rearrange("(h p) f -> p h f", p=P))
            nc.sync.dma_start(out=gb_sb, in_=gb_d[:, :])
            nc.sync.dma_start(out=ebT_sb[:], in_=eb_d[:, :])

            for rep in range(repeats):
                # Pre-issue the first wet-tile DMAs (alternating HWDGE
                # rings) so the weight stream runs from t=0, overlapping
                # the whole gate phase.
                wet_pre = []
                for t in range(wet_bufs):
                    wet0 = wep.tile([P, d_chunk, E], f32, tag="we",
                                    name=f"wet_pre{rep}_{t}")
                    eng = nc.sync if t % 2 == 0 else nc.scalar
                    eng.dma_start(
                        out=wet0[:],
                        in_=ew_d[:, t * d_chunk:(t + 1) * d_chunk, :])
                    wet_pre.append(wet0)

                # ============ Phase 1a: x -> xT (PE transposes) ========
                with tc.tile_pool(name=f"tpsum{rep}", bufs=2,
                                  space="PSUM") as tpsum, \
                     tc.tile_pool(name=f"gpsum{rep}", bufs=2,
                                  space="PSUM") as gpsum:

                    # PE touch: absorb gpsimd tick (identity)
                    nc.tensor.matmul(dummy[:], ident[:, 0:1], ident[:, 0:1],
                                     start=True, stop=True)

                    def do_chunk(a):
                        xc = xcp.tile([P, D], f32, tag="xc",
                                      name=f"xc{rep}_{a}")
                        nc.scalar.dma_start(
                            out=xc[:], in_=x_d[a * P:(a + 1) * P, :])
                        # PE touch absorbs this chunk's DMA tick
                        nc.tensor.matmul(dummy[:], xc[:, 0:1], xc[:, 0:1],
                                         start=True, stop=True)
                        for dh in range(DH):
                            pt = tpsum.tile([P, P], f32, tag="pt",
                                            name=f"pt{rep}_{a}_{dh}")
                            nc.tensor.transpose(
                                pt[:], xc[:, dh * P:(dh + 1) * P], ident)
                            nc.scalar.copy(
                                xT[:, dh, a * P:(a + 1) * P], pt[:])

                    # gate needs only chunks 0..7; transpose those first,
                    # interleave the other 24 with the gate MM groups
                    for a in range(EBLK):
                        do_chunk(a)

                    # ============ Phase 1b: gate + softmax =============
                    for a in range(EBLK):
                        for extra in range(3):
                            do_chunk(EBLK + a * 3 + extra)
                        lp = gpsum.tile([P, F], f32, tag="lp")
                        for half in range(2):
                            sl = slice(half * 512, (half + 1) * 512)
                            nc.tensor.matmul(lp[:, sl],
                                             xT[:, 0, a * P:(a + 1) * P],
                                             gw_sb[:, 0, sl],
                                             start=True, stop=False)
                            nc.tensor.matmul(lp[:, sl],
                                             xT[:, 1, a * P:(a + 1) * P],
                                             gw_sb[:, 1, sl],
                                             start=False, stop=False)
                            nc.tensor.matmul(lp[:, sl], ones_r128,
                                             gb_sb[:, sl],
                                             start=False, stop=True)
                        if a >= 1:
                            # ACT touch: absorb ts_mul(a-1)'s DVE tick
                            nc.scalar.copy(act_scr, wnorm[0:1, a - 1, 0:1])
                        nc.scalar.activation(expsc[:], lp[:], AF.Exp,
                                             accum_out=rsum)
                        nc.vector.reciprocal(rcp, rsum)
                        nc.vector.tensor_scalar_mul(
                            wnorm[:, a, :], expsc[:, 0:FSH], rcp)

                    # ====== Phase 1c: wnT (transpose w_norm) + b_eff ===
                    for a in range(EBLK):
                        pt = tpsum.tile([P, P], f32, tag="pt")
                        nc.tensor.transpose(pt[:], wnorm[:, a, :], ident)
                        nc.scalar.copy(
                            wnT[:, a * P:(a + 1) * P], pt[:])
                    nc.vector.scalar_tensor_tensor(
                        out=junk[:], in0=ebT_sb[:], scalar=1.0,
                        in1=wnT[:], op0=mult, op1=mult,
                        accum_out=beff_col)

                # ============ Phase 2: W_eff^T on the vector engine ====
                for t in range(NDT):
                    if t < wet_bufs:
                        wet = wet_pre[t]
                    else:
                        wet = wep.tile([P, d_chunk, E], f32, tag="we")
                        eng = nc.sync if t % 2 == 0 else nc.scalar
                        eng.dma_start(
                            out=wet[:],
                            in_=ew_d[:, t * d_chunk:(t + 1) * d_chunk, :])
                    for di in range(d_chunk):
                        d = t * d_chunk + di
                        nc.vector.scalar_tensor_tensor(
                            out=junk[:], in0=wet[:, di, :], scalar=1.0,
                            in1=wnT[:], op0=mult, op1=mult,
                            accum_out=wefft[:, d:d + 1])

                # ====== Phase 3: W_eff transpose + final GEMM ==========
                with tc.tile_pool(name=f"fpsum{rep}", bufs=2,
                                  space="PSUM") as fpsum:
                    for dh in range(DH):
                        pt = fpsum.tile([P, P], f32, tag="pt3")
                        nc.tensor.transpose(
                            pt[:], wefft[:, dh * P:(dh + 1) * P], ident)
                        nc.scalar.copy(
                            weff[:, dh * FSH:(dh + 1) * FSH], pt[:])
                    for ch in range(N // 512):
                        sl = slice(ch * 512, (ch + 1) * 512)
                        ps = fpsum.tile([P, 512], f32, tag="fp")
                        nc.tensor.matmul(ps[:], weff[:, 0:FSH],
                                         xT[:, 0, sl],
                                         start=True, stop=False)
                        nc.tensor.matmul(ps[:], weff[:, FSH:2 * FSH],
                                         xT[:, 1, sl],
                                         start=False, stop=True)
                        # psum->SBUF copy with per-partition bias add
                        nc.scalar.activation(outT_sb[:, sl], ps[:],
                                             AF.Identity, bias=beff_col,
                                             scale=1.0)
                        nc.sync.dma_start(out=out_d[:, sl],
                                          in_=outT_sb[:, sl])

    _split_multi_waits(nc)
    return nc


def _prep_in_maps(x, gate_W, gate_b, expert_W, expert_b):
    x = np.ascontiguousarray(np.asarray(x, dtype=np.float32))
    gate_W = np.asarray(gate_W, dtype=np.float32)
    gate_b = np.asarray(gate_b, dtype=np.float32).reshape(1, F)
    expert_W = np.asarray(expert_W, dtype=np.float32)
    expert_b = np.asarray(expert_b, dtype=np.float32)

    in_maps = []
    for c in range(NCORES):
        sh = slice(c * FSH, (c + 1) * FSH)
        in_maps.append({
            "x": x,
            # roll shard columns to the front; softmax row-sum is invariant
            "gw": np.ascontiguousarray(np.roll(gate_W, -c * FSH, axis=1)),
            "gb": np.ascontiguousarray(np.roll(gate_b, -c * FSH, axis=1)),
            # [E, D, FSH] -> [FSH, D, E]
            "ewt": np.ascontiguousarray(
                expert_W[:, :, sh].transpose(2, 1, 0)),
            "ebt": np.ascontiguousarray(expert_b[:, sh].T),
        })
    return in_maps


def kernel(x, gate_W, gate_b, expert_W, expert_b, _trace=False):
    global LAST_RESULT
    from concourse.bass_utils import run_bass_kernel_spmd

    if "nc" not in _CACHE:
        _CACHE["nc"] = _build_bass()
    nc = _CACHE["nc"]

    in_maps = _prep_in_maps(x, gate_W, gate_b, expert_W, expert_b)

    res = run_bass_kernel_spmd(
        nc, in_maps, core_ids=list(range(NCORES)), trace=_trace,
    )
    LAST_RESULT = res

    out = np.empty([N, F], dtype=np.float32)
    for c in range(NCORES):
        out[:, c * FSH:(c + 1) * FSH] = res.results[c]["outT"].T
    return out

